# revision 6
# baseline (speedup 1.0000x reference)
"""BiLSTM (reference nn_CharBiGRU) Trainium2 Bass kernel, v3.

Distribution: 8 cores = 4 batch-groups (16 rows) x 2 directions; each core
runs ONE direction's LSTM over its 16 batch rows. Vs v2 (2 dirs/core, 8
rows) this halves the PE instruction count per step - the measured
bottleneck is PE sequencer dispatch + semaphore chains, not engine
throughput - and halves per-core weight streaming.

  Phase A (GEMM): Xi[t,b,:] = x[b,t,:] @ Wi.T (+bias) for ALL t as dense
    fp32r matmuls (full-rate streaming), PSUM [128=(8t x 16b), 512-strip],
    evacuated with a DVE bias-add + bf16 cast to internal-DRAM Xi.

  Phase B (recurrence), per step: Xi_t injected into PSUM via an
    identity-stationary matmul ([I16|0], M=32 covers the full strip rows);
    4 K-chunk bf16 Wh.T matmuls per gate strip, col-tiled (0,32s); one
    sigmoid ACT over all strips (g pre-scaled x2: tanh(g)=2*sigmoid(2g)-1,
    fixed up on DVE); cell math split DVE/GPSIMD; h back to stationary
    layout via 4 PE transposes + DVE copy.
"""

import numpy as np

B, T, D, H = 64, 512, 512, 512
G4 = 4 * H
NCORES = 8
NG = 4            # batch groups
BL = B // NG      # 16 rows per core
TPM = 128 // BL   # 8 timesteps per GEMM m-chunk

_CACHE = {}


def build_kernel(T_steps=T):
    import concourse.bass as bass
    import concourse.bacc as bacc
    import concourse.mybir as mybir
    from concourse.tile import TileContext

    fp32 = mybir.dt.float32
    f32r = mybir.dt.float32r
    bf16 = mybir.dt.bfloat16
    AF = mybir.ActivationFunctionType
    ALU = mybir.AluOpType

    NM = (T_steps * BL) // 128  # m-chunks of (8 t x 16 b)

    nc = bacc.Bacc()
    xg = nc.declare_dram_parameter("xg", [4, 128, T_steps * BL], f32r, isOutput=False)
    wit = nc.declare_dram_parameter("wit", [4, 128, G4], f32r, isOutput=False)
    wht = nc.declare_dram_parameter("wht", [4, 128, G4], bf16, isOutput=False)
    brow = nc.declare_dram_parameter("brow", [1, G4], fp32, isOutput=False)
    identp = nc.declare_dram_parameter("identp", [16, 32], bf16, isOutput=False)
    h0t = nc.declare_dram_parameter("h0t", [4, 128, BL], bf16, isOutput=False)
    c0 = nc.declare_dram_parameter("c0", [BL, H], fp32, isOutput=False)
    ys = nc.declare_dram_parameter("ys", [T_steps, BL, H], bf16, isOutput=True)

    with TileContext(nc) as tc:
        with (
            tc.tile_pool(name="persist", bufs=1) as pp,
            tc.tile_pool(name="dramp", bufs=1, space="DRAM") as dp,
        ):
            ident = pp.tile([16, 32], bf16, name="ident")
            nc.sync.dma_start(out=ident[:, :], in_=identp[:, :])

            whk = [pp.tile([128, G4], bf16, name=f"wh{k}") for k in range(4)]
            for k in range(4):
                nc.sync.dma_start(out=whk[k][:, :], in_=wht[k])

            brow_sb = pp.tile([1, G4], fp32, name="brow_sb")
            nc.sync.dma_start(out=brow_sb[0:1, :], in_=brow[:, :])
            bias_bc = pp.tile([128, G4], fp32, name="bias_bc")
            nc.gpsimd.partition_broadcast(bias_bc[:, :], brow_sb[0:1, :])

            hT = pp.tile([128, 4 * BL], bf16, name="hT")
            Ct = pp.tile([32 + BL, H], fp32, name="Ct")
            for k in range(4):
                nc.sync.dma_start(out=hT[:, BL * k:BL * (k + 1)], in_=h0t[k])
            nc.sync.dma_start(out=Ct[32:32 + BL, :], in_=c0[:, :])

            xi_dram = dp.tile([NM, 128, G4], bf16, name="xi_dram")

            # ---------------- Phase A: Xi GEMM ----------------
            with (
                tc.tile_pool(name="witp", bufs=1) as witp,
                tc.tile_pool(name="xkp", bufs=8) as xkp,
                tc.tile_pool(name="gemmps", bufs=4, space="PSUM") as gemmps,
                tc.tile_pool(name="stagep", bufs=3) as stagep,
            ):
                wit_sb = [witp.tile([128, G4], f32r, name=f"wi{k}") for k in range(4)]
                for k in range(4):
                    nc.sync.dma_start(out=wit_sb[k][:, :], in_=wit[k])

                for m in range(NM):
                    xk = [xkp.tile([128, 128], f32r, tag=f"xk{k}", name=f"xk{m}{k}")
                          for k in range(4)]
                    for k in range(4):
                        nc.sync.dma_start(
                            out=xk[k][:, :], in_=xg[k, :, 128 * m:128 * (m + 1)])
                    st = stagep.tile([128, G4], bf16, tag="st", name=f"st{m}")
                    for s in range(4):
                        P = gemmps.tile([128, 512], fp32, tag="P", name=f"P{m}{s}")
                        for k in range(4):
                            nc.tensor.matmul(
                                P[:, :], xk[k][:, :],
                                wit_sb[k][:, 512 * s:512 * (s + 1)],
                                start=(k == 0), stop=(k == 3),
                            )
                        nc.vector.scalar_tensor_tensor(
                            out=st[:, 512 * s:512 * (s + 1)],
                            in0=P[:, :], scalar=0.0,
                            in1=bias_bc[:, 512 * s:512 * (s + 1)],
                            op0=ALU.add, op1=ALU.add,
                        )
                    nc.sync.dma_start(out=xi_dram[m], in_=st[:, :])

            # ---------------- Phase B: recurrence ----------------
            with (
                tc.tile_pool(name="xip", bufs=6) as xip,
                tc.tile_pool(name="gps", bufs=2, space="PSUM") as gps,
                tc.tile_pool(name="ptp", bufs=1, space="PSUM") as ptp,
                tc.tile_pool(name="workp", bufs=2) as workp,
            ):
                for t in range(T_steps):
                    xi_t = xip.tile([BL, G4], bf16, tag="xi", name=f"xi_{t}")
                    mm, r = divmod(t, TPM)
                    nc.sync.dma_start(
                        out=xi_t[:, :], in_=xi_dram[mm, BL * r:BL * (r + 1), :])

                    G = gps.tile([128, 512], fp32, tag="G", name=f"G_{t}")
                    for s in range(4):
                        tp = (0, 32 * s)
                        nc.tensor.matmul(
                            G[32 * s:32 * s + 32, :], ident[0:BL, 0:32],
                            xi_t[0:BL, 512 * s:512 * (s + 1)],
                            start=True, stop=False, tile_position=tp,
                            skip_group_check=True,
                        )
                        outs = G[32 * s:32 * s + BL, :]
                        for k in range(4):
                            nc.tensor.matmul(
                                outs, hT[:, BL * k:BL * (k + 1)],
                                whk[k][:, 512 * s:512 * (s + 1)],
                                start=False, stop=(k == 3), tile_position=tp,
                                skip_group_check=True,
                            )

                    Y = workp.tile([96 + BL, 512], bf16, tag="Y", name=f"Y_{t}")
                    nc.scalar.activation(Y[:, :], G[0:96 + BL, :], AF.Sigmoid)

                    TG = workp.tile([BL, 512], bf16, tag="TG", name=f"TG_{t}")
                    nc.vector.tensor_scalar(
                        TG[:, :], Y[96:96 + BL, :], 2.0, -1.0, ALU.mult, ALU.add)
                    U = workp.tile([BL, 512], bf16, tag="U", name=f"U_{t}")
                    nc.gpsimd.tensor_mul(U[:, :], Y[0:BL, :], TG[:, :])
                    V = workp.tile([BL, 512], fp32, tag="V", name=f"V_{t}")
                    nc.vector.tensor_mul(V[:, :], Y[32:32 + BL, :], Ct[32:32 + BL, :])
                    nc.vector.tensor_add(Ct[32:32 + BL, :], U[:, :], V[:, :])
                    Z = workp.tile([64 + BL, 512], bf16, tag="Z", name=f"Z_{t}")
                    nc.scalar.activation(Z[64:64 + BL, :], Ct[32:32 + BL, :], AF.Tanh)
                    ht = workp.tile([BL, 512], bf16, tag="h", name=f"h_{t}")
                    nc.gpsimd.tensor_mul(ht[:, :], Y[64:64 + BL, :], Z[64:64 + BL, :])

                    nc.sync.dma_start(out=ys[t], in_=ht[:, :])

                    PT = ptp.tile([128, 4 * BL], bf16, tag="PT", name=f"PT_{t}")
                    for k in range(4):
                        nc.tensor.transpose(
                            PT[:, BL * k:BL * (k + 1)], ht[:, 128 * k:128 * (k + 1)],
                            ident[0:BL, 0:BL])
                    nc.vector.tensor_copy(hT[:, :], PT[:, :])

    nc.finalize()
    return nc


def _host_prep(inputs_emb, mask, h0, c0, Wi_f, Wh_f, b_f, Wi_b, Wh_b, b_b):
    import ml_dtypes
    bf16 = ml_dtypes.bfloat16

    x = np.asarray(inputs_emb, dtype=np.float32)
    mask = np.asarray(mask, dtype=np.float32)
    lengths = mask.astype(np.int32).sum(axis=1)  # [B]
    t_idx = np.arange(T, dtype=np.int64)[None, :]
    P = (lengths[:, None].astype(np.int64) - 1 - t_idx) % T  # [B, T] involution
    x_proc = np.take_along_axis(x, P[:, :, None], axis=1)  # [B, T, D]

    # device strip order (i, f, o, g); reference is (i, f, g, o).
    # g strip pre-scaled x2: tanh(g) = 2*sigmoid(2g) - 1.
    PERM = [0, 1, 3, 2]
    GSCALE = np.array([1.0, 1.0, 1.0, 2.0], np.float32)[:, None, None]

    def chunks(W, dt):
        W = np.asarray(W, dtype=np.float32)
        Wp = (W.reshape(4, H, -1)[PERM] * GSCALE).reshape(G4, -1)
        Wt = np.ascontiguousarray(Wp.T)
        return Wt.reshape(4, 128, G4).astype(dt)

    def pbias(b):
        return (np.asarray(b, np.float32).reshape(4, H)[PERM]
                * GSCALE[:, :, 0]).reshape(1, G4)

    wi = [chunks(Wi_f, np.float32), chunks(Wi_b, np.float32)]
    wh = [chunks(Wh_f, bf16), chunks(Wh_b, bf16)]
    br = [pbias(b_f).astype(np.float32), pbias(b_b).astype(np.float32)]
    xs = [x, x_proc]

    identp = np.zeros((16, 32), np.float32)
    identp[np.arange(16), np.arange(16)] = 1.0
    identp = identp.astype(bf16)

    h0 = np.asarray(h0, np.float32)
    c0 = np.asarray(c0, np.float32)

    in_maps = []
    for cidx in range(NCORES):
        d, g = divmod(cidx, NG)
        sl = slice(g * BL, (g + 1) * BL)
        xd = xs[d][sl]  # [BL, T, D]
        # xg[k, p, t*BL+b] = x[b, t, 128k+p]
        xgc = np.ascontiguousarray(xd.transpose(2, 1, 0).reshape(4, 128, T * BL))
        h0c = h0[sl]
        in_maps.append({
            "xg": xgc, "wit": wi[d], "wht": wh[d], "brow": br[d],
            "identp": identp,
            "h0t": np.ascontiguousarray(h0c.T.reshape(4, 128, BL)).astype(bf16),
            "c0": np.ascontiguousarray(c0[sl]),
        })
    return in_maps, P


def _host_post(results, P):
    ys_f = np.concatenate(
        [results[g]["ys"].transpose(1, 0, 2).astype(np.float32)
         for g in range(NG)], 0)
    ys_b = np.concatenate(
        [results[NG + g]["ys"].transpose(1, 0, 2).astype(np.float32)
         for g in range(NG)], 0)
    out_b = np.take_along_axis(ys_b, P[:, :, None], axis=1)
    return np.concatenate([ys_f, out_b], axis=-1).astype(np.float32)


def kernel(**inputs):
    from concourse.bass_utils import run_bass_kernel_spmd
    in_maps, P = _host_prep(**inputs)
    if "nc" not in _CACHE:
        _CACHE["nc"] = build_kernel()
    nc = _CACHE["nc"]
    res = run_bass_kernel_spmd(nc, in_maps, list(range(NCORES)))
    return _host_post(res.results, P)
